# revision 25
# baseline (speedup 1.0000x reference)
"""CrossViewAttention Trainium2 kernel, v3.

Sharding: Q=2500 queries split across 8 cores (QC=320 each, padded).
Softmax is over NK (local per core) so no collectives.

Per core, in the transposed attention layout (nk on partitions, q free):

- k arrives column-major [D, NK]; the k-side projection is a plain
  matmul kf = WkC^T-proj(kT) (13 N=512 matmuls + PSUM->SBUF copies),
  where WkC is row-centered on the host so the k-LayerNorm mean
  subtraction is exact.  The k-LN 1/std rides the ACT engine's
  per-partition `scale` operand of the exp.  Logits then contract only
  DH=32: two heads run CONCURRENTLY in the PE array via row-packing.
- v arrives row-major [NK, D] (lhsT of the attention@V matmuls, in raw
  d-space) plus column-major [D, NK] for stats; its LayerNorm applies
  as vS = vR*rstd - m*rstd on GPSIMD.  The fused (wproj_h @ wv_h)
  matrix maps raw d-space to the output at the end.
- The `* vis` mask multiply after the exp is eliminated exactly via
  exp(x)*vis = ee - 1 + vis: the "-1+vis" part is one PE matmul per
  tile against the host tensor (vis-1) plus a host denominator
  correction.
- Denominators accumulate via M=1 ones-matmuls col-packed into one
  PSUM bank; heads run in two passes of two so PSUM fits exactly.
- A burst of dummy matmuls at program start trips the PE HAM clock
  gate early so the whole kernel runs at 2.4 GHz instead of 1.2.
- gelu is an odd polynomial on GPSIMD; every LayerNorm 1/sqrt is
  exp(-0.5*ln(var+eps)) so only exp/ln/square/identity ACT functions
  are used.
"""

import sys

if "/opt/trn_rl_repo" not in sys.path:
    sys.path.insert(0, "/opt/trn_rl_repo")

import numpy as np
import ml_dtypes

import concourse.bass as bass
import concourse.bacc as bacc_mod
import concourse.mybir as mybir
from concourse.tile import TileContext

HEADS = 4
DH = 32
D = 128
EPS = 1e-5
HB = WB = 50
Q = HB * WB            # 2500
NVIEW, KH, KW = 6, 24, 44
NK = NVIEW * KH * KW   # 6336
NCORES = 8
QC = 320               # queries per core (Q padded to 2560)
QPAD = NCORES * QC
NKP = 6400             # NK padded to 50*128
NKT = NKP // 128       # 50 nk tiles
SCALE = DH ** -0.5

F32 = mybir.dt.float32
BF16 = mybir.dt.bfloat16
AF = mybir.ActivationFunctionType
ALU = mybir.AluOpType

_CACHE = {}


def _fit_gelu():
    import math as _m
    xs = np.linspace(-2.8, 2.8, 4001)
    xs = xs[np.abs(xs) > 1e-3]
    phi = np.array([0.5 * (1.0 + _m.erf(t / _m.sqrt(2.0))) for t in xs])
    z = (phi - 0.5) / xs
    y = xs * xs
    Amat = np.stack([np.ones_like(y), y, y * y], axis=1)
    coef, *_ = np.linalg.lstsq(Amat, z, rcond=None)
    return [float(c) for c in coef]

GA, GB, GC = _fit_gelu()


def _ln_partition(nc, work, ps_pool, ones_col, ones_row, eps_c, x, out,
                  g_ap, b_ap):
    """LayerNorm over the PARTITION dim of x [128, Qf] -> out."""
    Qf = x.shape[-1]
    sq = work.tile([128, Qf], F32, tag="ln_sq")
    nc.scalar.activation(sq, x, AF.Square)
    s1 = ps_pool.tile([1, Qf], F32, tag="ln_s")
    nc.tensor.matmul(s1, ones_col, x, start=True, stop=True)
    mean = work.tile([1, Qf], F32, tag="ln_mean")
    nc.scalar.mul(mean, s1, 1.0 / 128.0)
    s2 = ps_pool.tile([1, Qf], F32, tag="ln_s")
    nc.tensor.matmul(s2, ones_col, sq, start=True, stop=True)
    ex2 = work.tile([1, Qf], F32, tag="ln_ex2")
    nc.scalar.mul(ex2, s2, 1.0 / 128.0)
    m2 = work.tile([1, Qf], F32, tag="ln_m2")
    nc.vector.tensor_mul(out=m2, in0=mean, in1=mean)
    var = work.tile([1, Qf], F32, tag="ln_var")
    nc.vector.tensor_tensor(out=var, in0=ex2, in1=m2, op=ALU.subtract)
    lnv = work.tile([1, Qf], F32, tag="ln_lnv")
    nc.scalar.activation(lnv, var, AF.Ln, bias=eps_c[0:1])
    rstd = work.tile([1, Qf], F32, tag="ln_rstd")
    nc.scalar.activation(rstd, lnv, AF.Exp, scale=-0.5)
    nmr = work.tile([1, Qf], F32, tag="ln_nmr")
    nc.vector.tensor_mul(out=nmr, in0=mean, in1=rstd)
    nc.scalar.mul(nmr, nmr, -1.0)
    rstdB = ps_pool.tile([128, Qf], F32, tag="ln_b")
    nc.tensor.matmul(rstdB, ones_row, rstd, start=True, stop=True)
    t1 = work.tile([128, Qf], F32, tag="ln_t1")
    nc.vector.tensor_mul(out=t1, in0=x, in1=rstdB)
    nmrB = ps_pool.tile([128, Qf], F32, tag="ln_b")
    nc.tensor.matmul(nmrB, ones_row, nmr, start=True, stop=True)
    if g_ap is None:
        nc.vector.tensor_add(out=out, in0=t1, in1=nmrB)
    else:
        t2 = work.tile([128, Qf], F32, tag="ln_t2")
        nc.vector.tensor_add(out=t2, in0=t1, in1=nmrB)
        nc.scalar.activation(out, t2, AF.Identity, scale=g_ap, bias=b_ap)


def _build():
    if "nc" in _CACHE:
        return _CACHE["nc"]
    nc = bacc_mod.Bacc()

    # ---- I/O ----
    qT = nc.dram_tensor("qT", [D, QC], F32, kind="ExternalInput")
    skipT = nc.dram_tensor("skipT", [D, QC], F32, kind="ExternalInput")
    kTd = nc.dram_tensor("kTd", [D, NKP], BF16, kind="ExternalInput")
    kRd = nc.dram_tensor("kRd", [NKP, D], BF16, kind="ExternalInput")
    vRd = nc.dram_tensor("vRd", [NKP, D], BF16, kind="ExternalInput")
    Wvd = nc.dram_tensor("Wvd", [NKT, 128, QC], BF16, kind="ExternalInput")
    Vm1d = nc.dram_tensor("Vm1d", [NKT, 128, QC], BF16, kind="ExternalInput")
    dcord = nc.dram_tensor("dcord", [D, QC], F32, kind="ExternalInput")
    Wq2Td = nc.dram_tensor("Wq2Td", [D, D], BF16, kind="ExternalInput")
    bq2d = nc.dram_tensor("bq2d", [D, 1], F32, kind="ExternalInput")
    WkCTd = nc.dram_tensor("WkCTd", [D, D], BF16, kind="ExternalInput")
    WCTd = nc.dram_tensor("WCTd", [D, HEADS, D], BF16, kind="ExternalInput")
    bprojd = nc.dram_tensor("bprojd", [D, 1], F32, kind="ExternalInput")
    pre_gd = nc.dram_tensor("pre_gd", [D, 1], F32, kind="ExternalInput")
    pre_bd = nc.dram_tensor("pre_bd", [D, 1], F32, kind="ExternalInput")
    w1Td = nc.dram_tensor("w1Td", [D, 2 * D], BF16, kind="ExternalInput")
    b1d = nc.dram_tensor("b1d", [D, 2], F32, kind="ExternalInput")
    w2Tdd = nc.dram_tensor("w2Tdd", [2, D, D], BF16, kind="ExternalInput")
    b2d = nc.dram_tensor("b2d", [D, 1], F32, kind="ExternalInput")
    post_gd = nc.dram_tensor("post_gd", [D, 1], F32, kind="ExternalInput")
    post_bd = nc.dram_tensor("post_bd", [D, 1], F32, kind="ExternalInput")
    outT = nc.dram_tensor("outT", [D, QC], F32, kind="ExternalOutput")

    with TileContext(nc) as tc:
        with tc.tile_pool(name="const", bufs=1) as cpool, \
             tc.tile_pool(name="big", bufs=1) as bigpool, \
             tc.tile_pool(name="work", bufs=1) as work, \
             tc.tile_pool(name="io", bufs=1) as io:

            # ---- constants ----
            ones_col = cpool.tile([128, 1], F32)
            nc.any.memset(ones_col, 1.0)
            ones_row = cpool.tile([1, 128], F32)
            nc.any.memset(ones_row, 1.0)
            ones_rows_f = cpool.tile([128, 128], F32)
            nc.any.memset(ones_rows_f, 1.0)
            onesb_col = cpool.tile([128, 1], BF16)
            nc.any.memset(onesb_col, 1.0)
            invd_col = cpool.tile([128, 1], BF16)
            nc.any.memset(invd_col, 1.0 / 128.0)
            zero_c = cpool.tile([128, 1], F32)
            nc.any.memset(zero_c, 0.0)
            nc.const_aps.aps[(F32, 0.0)] = zero_c[:]
            eps_c = cpool.tile([128, 1], F32)
            nc.any.memset(eps_c, EPS)
            nc.const_aps.aps[(F32, EPS)] = eps_c[:]
            magic_c = cpool.tile([128, 1], mybir.dt.int32)
            nc.any.memset(magic_c, 0x5F3759DF)
            warm_w = cpool.tile([128, 128], BF16)
            nc.any.memset(warm_w, 1.0)
            warm_x = cpool.tile([128, 512], BF16)
            nc.any.memset(warm_x, 1.0)

            def load_const(dram, shape, dt):
                t = cpool.tile(shape, dt, tag="c_" + dram.name)
                nc.sync.dma_start(t, dram[...])
                return t

            Wq2T_s = load_const(Wq2Td, [D, D], BF16)
            bq2_s = load_const(bq2d, [D, 1], F32)
            WkCT_s = load_const(WkCTd, [D, D], BF16)
            WCT_s = load_const(WCTd, [D, HEADS, D], BF16)
            bproj_s = load_const(bprojd, [D, 1], F32)
            preg_s = load_const(pre_gd, [D, 1], F32)
            preb_s = load_const(pre_bd, [D, 1], F32)
            w1_s = load_const(w1Td, [D, 2 * D], BF16)
            b1_s = load_const(b1d, [D, 2], F32)
            w2_s = cpool.tile([D, 2, D], BF16)
            nc.sync.dma_start(w2_s[:, 0, :], w2Tdd[0])
            nc.sync.dma_start(w2_s[:, 1, :], w2Tdd[1])
            b2_s = load_const(b2d, [D, 1], F32)
            postg_s = load_const(post_gd, [D, 1], F32)
            postb_s = load_const(post_bd, [D, 1], F32)
            dcor_s = load_const(dcord, [D, QC], F32)

            # ---- persistent tensors ----
            kf_sb = bigpool.tile([128, NKP], BF16)     # projected k (inner-major)
            vS_s = bigpool.tile([128, NKT, D], BF16)   # LayerNormed v (row-major)
            Wv_s = bigpool.tile([128, NKT, QC], BF16)
            Vm1_s = bigpool.tile([128, NKT, QC], BF16)
            qf_sb = bigpool.tile([D, QC], BF16)
            rstdk = bigpool.tile([128, NKT], F32)
            A_sb = bigpool.tile([128, HEADS, QC], BF16)
            B_sb = bigpool.tile([128, QC], BF16)

            # ---- prep ----
            with tc.tile_pool(name="kvsrc", bufs=1) as kvsrc, \
                 tc.tile_pool(name="ps_prep", bufs=1, space="PSUM") as psp, \
                 tc.tile_pool(name="ps_kf", bufs=2, space="PSUM") as pskf, \
                 tc.tile_pool(name="prepw", bufs=1) as prepw, \
                 tc.tile_pool(name="pipe", bufs=2) as pipe:

                # PE warm-up burst (trips the HAM clock gate to 2.4 GHz)
                warm_ps = psp.tile([128, 512], F32, tag="warm")
                for _ in range(32):
                    nc.tensor.matmul(warm_ps, warm_w, warm_x,
                                     start=True, stop=True)

                KCH = 1280
                kT_s = kvsrc.tile([128, NKP], BF16, tag="kT")
                kR_s = kvsrc.tile([128, NKT, D], BF16, tag="kR")
                vR_s = kvsrc.tile([128, NKT, D], BF16, tag="vR")
                VCH = 10
                for c0 in range(0, NKT, VCH):
                    nc.scalar.dma_start(
                        kR_s[:, c0:c0 + VCH, :],
                        kRd[c0 * 128:(c0 + VCH) * 128, :].rearrange(
                            "(t p) d -> p t d", p=128))
                for c in range(0, NKP, KCH):
                    nc.scalar.dma_start(kT_s[:, c:c + KCH], kTd[:, c:c + KCH])
                for c0 in range(0, NKT, VCH):
                    nc.gpsimd.dma_start(
                        vR_s[:, c0:c0 + VCH, :],
                        vRd[c0 * 128:(c0 + VCH) * 128, :].rearrange(
                            "(t p) d -> p t d", p=128))
                for t in range(NKT):
                    nc.sync.dma_start(Wv_s[:, t, :], Wvd[t])
                    nc.gpsimd.dma_start(Vm1_s[:, t, :], Vm1d[t])

                # ---- k/v stats per 10-tile chunk: bn_stats + quake rsqrt
                # (v first each chunk so the gpsimd vS pass starts early)
                rstdv = prepw.tile([128, NKT], F32, tag="rstdv")
                mtil = prepw.tile([128, NKT], F32, tag="mtil")

                def chunk_stats(srcR, c0, rstd_out, mtil_out):
                    cs = slice(c0, c0 + VCH)
                    bs = prepw.tile([128, VCH, 6], F32, tag="bs")
                    for t in range(c0, c0 + VCH):
                        nc.vector.bn_stats(bs[:, t - c0, :], srcR[:, t, :])
                    me, mo = bs[:, :, 1], bs[:, :, 4]
                    m2e, m2o = bs[:, :, 2], bs[:, :, 5]
                    mean = prepw.tile([128, VCH], F32, tag="cmean")
                    nc.vector.tensor_add(out=mean, in0=me, in1=mo)
                    nc.scalar.mul(mean, mean, 0.5)
                    tee = prepw.tile([128, VCH], F32, tag="ctee")
                    nc.vector.tensor_mul(out=tee, in0=me, in1=me)
                    too = prepw.tile([128, VCH], F32, tag="ctoo")
                    nc.vector.tensor_mul(out=too, in0=mo, in1=mo)
                    sum2 = prepw.tile([128, VCH], F32, tag="csum2")
                    nc.vector.tensor_add(out=sum2, in0=tee, in1=too)
                    m2s = prepw.tile([128, VCH], F32, tag="cm2s")
                    nc.vector.tensor_add(out=m2s, in0=m2e, in1=m2o)
                    ex2 = prepw.tile([128, VCH], F32, tag="cex2")
                    nc.vector.tensor_scalar(out=ex2, in0=m2s,
                                            scalar1=1.0 / 128.0,
                                            scalar2=None, op0=ALU.mult)
                    nc.scalar.mul(sum2, sum2, 0.5)
                    nc.vector.tensor_add(out=ex2, in0=ex2, in1=sum2)
                    mv2 = prepw.tile([128, VCH], F32, tag="cmv2")
                    nc.vector.tensor_mul(out=mv2, in0=mean, in1=mean)
                    var = prepw.tile([128, VCH], F32, tag="cvar")
                    nc.vector.tensor_tensor(out=var, in0=ex2, in1=mv2,
                                            op=ALU.subtract)
                    nc.vector.tensor_scalar(out=var, in0=var, scalar1=EPS,
                                            scalar2=None, op0=ALU.add)
                    # quake rsqrt + two Newton steps (DVE only)
                    vi = var.bitcast(mybir.dt.int32)
                    sh = prepw.tile([128, VCH], mybir.dt.int32, tag="csh")
                    nc.vector.tensor_scalar(out=sh, in0=vi, scalar1=1,
                                            scalar2=None,
                                            op0=ALU.logical_shift_right)
                    y0i = prepw.tile([128, VCH], mybir.dt.int32, tag="cy0")
                    nc.vector.tensor_tensor(
                        out=y0i,
                        in0=magic_c[:, 0:1].to_broadcast((128, VCH)),
                        in1=sh, op=ALU.subtract)
                    y = y0i.bitcast(F32)
                    for it in range(2):
                        n1 = prepw.tile([128, VCH], F32, tag=f"cn1_{it}")
                        nc.vector.tensor_mul(out=n1, in0=y, in1=y)
                        nc.vector.tensor_mul(out=n1, in0=n1, in1=var)
                        nc.vector.tensor_scalar(out=n1, in0=n1, scalar1=-0.5,
                                                scalar2=1.5, op0=ALU.mult,
                                                op1=ALU.add)
                        yn = prepw.tile([128, VCH], F32, tag=f"cyn_{it}")
                        nc.vector.tensor_mul(out=yn, in0=y, in1=n1)
                        y = yn
                    nc.vector.tensor_copy(out=rstd_out[:, cs], in_=y)
                    if mtil_out is not None:
                        nc.vector.tensor_mul(out=mtil_out[:, cs], in0=mean,
                                             in1=y)

                for c0 in range(0, NKT, VCH):
                    chunk_stats(vR_s, c0, rstdv, mtil)
                    chunk_stats(kR_s, c0, rstdk, None)
                    # vS chunk on gpsimd right away
                    tmpv = pipe.tile([128, VCH, D], BF16, tag="tmpv")
                    nc.gpsimd.tensor_tensor(
                        out=tmpv, in0=vR_s[:, c0:c0 + VCH, :],
                        in1=rstdv[:, c0:c0 + VCH, None].to_broadcast(
                            (128, VCH, D)), op=ALU.mult)
                    nc.gpsimd.tensor_tensor(
                        out=vS_s[:, c0:c0 + VCH, :], in0=tmpv,
                        in1=mtil[:, c0:c0 + VCH, None].to_broadcast(
                            (128, VCH, D)), op=ALU.subtract)

                # q LayerNorm stats (before the batched Ln/Exp era)
                qsb = io.tile([D, QC], F32, tag="qsb")
                nc.sync.dma_start(qsb, qT[...])
                qsq = work.tile([D, QC], F32, tag="qsq")
                nc.scalar.activation(qsq, qsb, AF.Square)
                qs1 = psp.tile([1, QC], F32, tag="qstat")
                nc.tensor.matmul(qs1, ones_col, qsb, start=True, stop=True)
                qmean = prepw.tile([1, QC], F32, tag="qmean")
                nc.scalar.mul(qmean, qs1, 1.0 / 128.0)
                qs2 = psp.tile([1, QC], F32, tag="qstat")
                nc.tensor.matmul(qs2, ones_col, qsq, start=True, stop=True)
                qex2 = prepw.tile([1, QC], F32, tag="qex2")
                nc.scalar.mul(qex2, qs2, 1.0 / 128.0)
                qm2 = prepw.tile([1, QC], F32, tag="qm2")
                nc.vector.tensor_mul(out=qm2, in0=qmean, in1=qmean)
                qvar = prepw.tile([1, QC], F32, tag="qvar")
                nc.vector.tensor_tensor(out=qvar, in0=qex2, in1=qm2,
                                        op=ALU.subtract)

                qlnv = prepw.tile([1, QC], F32, tag="qlnv")
                nc.scalar.activation(qlnv, qvar, AF.Ln, bias=eps_c[0:1])
                qrstd = prepw.tile([1, QC], F32, tag="qrstd")
                nc.scalar.activation(qrstd, qlnv, AF.Exp, scale=-0.5)

                # finish q LayerNorm + projection
                qnmr = prepw.tile([1, QC], F32, tag="qnmr")
                nc.vector.tensor_mul(out=qnmr, in0=qmean, in1=qrstd)
                nc.scalar.mul(qnmr, qnmr, -1.0)
                qrB = psp.tile([128, QC], F32, tag="qbc")
                nc.tensor.matmul(qrB, ones_row, qrstd, start=True, stop=True)
                qt1 = work.tile([D, QC], F32, tag="qt1")
                nc.vector.tensor_mul(out=qt1, in0=qsb, in1=qrB)
                qnB = psp.tile([128, QC], F32, tag="qbc")
                nc.tensor.matmul(qnB, ones_row, qnmr, start=True, stop=True)
                qn = io.tile([D, QC], BF16, tag="qn")
                nc.vector.tensor_add(out=qn, in0=qt1, in1=qnB)
                qfp = psp.tile([128, QC], F32, tag="qfp")
                nc.tensor.matmul(qfp, Wq2T_s, qn, start=True, stop=True)
                nc.scalar.activation(qf_sb, qfp, AF.Identity, bias=bq2_s)

                # k projection: kf = WkC-proj(kT), inner-major [128i, NKP]
                for c in range(0, NKP, 512):
                    cw = min(512, NKP - c)
                    kfp = pskf.tile([128, 512], F32, tag="kfp")
                    nc.tensor.matmul(kfp[:, 0:cw], WkCT_s, kT_s[:, c:c + cw],
                                     start=True, stop=True)
                    nc.scalar.copy(kf_sb[:, c:c + cw], kfp[:, 0:cw])

            # ---- attention: two passes of two heads (row-packed logits) ----
            with tc.tile_pool(name="ps_pl", bufs=2, space="PSUM") as ps_pl, \
                 tc.tile_pool(name="ps_acc", bufs=1, space="PSUM") as ps_acc, \
                 tc.tile_pool(name="ps_bd", bufs=1, space="PSUM") as ps_bd, \
                 tc.tile_pool(name="attw", bufs=3) as attw:
                den_t = ps_bd.tile([128, QC], F32, tag="den")
                ones_qc = work.tile([128, QC], F32, tag="ones_qc")
                nc.any.memset(ones_qc, 1.0)
                nc.scalar.copy(den_t, ones_qc)
                B_t = ps_bd.tile([128, QC], F32, tag="B")
                def emit_logits(hp, t):
                    pl = ps_pl.tile([128, 2, 512], F32, tag="pl")
                    for i in range(2):
                        h = 2 * hp + i
                        nc.tensor.matmul(
                            pl[:, i, 0:QC],
                            kf_sb[32 * h:32 * h + 32,
                                  t * 128:(t + 1) * 128],
                            qf_sb[32 * h:32 * h + 32, :],
                            start=True, stop=True,
                            tile_position=(32 * h, 0))
                    return pl

                for hp in range(2):
                    A01 = ps_acc.tile([128, 640], F32, tag="A01")
                    pl = emit_logits(hp, 0)
                    for t in range(NKT):
                        em = attw.tile([128, 2, QC], BF16, tag="em")
                        nc.vector.tensor_mul(
                            out=em, in0=pl[:, :, 0:QC],
                            in1=Wv_s[:, t, None, :].to_broadcast(
                                (128, 2, QC)))
                        ee = attw.tile([128, 2, QC], BF16, tag="ee")
                        nc.scalar.activation(ee, em, AF.Exp,
                                             scale=rstdk[:, t:t + 1])
                        if t < NKT - 1:
                            pl = emit_logits(hp, t + 1)
                        vsl = vS_s[:, t, :]
                        eef = ee[:].rearrange("p a b -> p (a b)")
                        nc.tensor.matmul(A01[:, 0:512], vsl, eef[:, 0:512],
                                         start=(t == 0), stop=(t == NKT - 1))
                        nc.tensor.matmul(A01[:, 512:640], vsl, eef[:, 512:640],
                                         start=(t == 0), stop=(t == NKT - 1))
                        if hp == 0:
                            nc.tensor.matmul(B_t, vsl, Vm1_s[:, t, :],
                                             start=(t == 0),
                                             stop=(t == NKT - 1))
                        for i in range(2):
                            h = 2 * hp + i
                            nc.tensor.matmul(den_t[32 * h:32 * h + 1, :],
                                             onesb_col, ee[:, i, :],
                                             start=(t == 0),
                                             stop=(t == NKT - 1),
                                             tile_position=(0, 32 * h),
                                             skip_group_check=True)
                    nc.scalar.copy(A_sb[:, 2 * hp, :], A01[:, 0:QC])
                    nc.scalar.copy(A_sb[:, 2 * hp + 1, :], A01[:, QC:2 * QC])
                    if hp == 0:
                        nc.scalar.copy(B_sb, B_t)

                den2 = io.tile([128, QC], F32, tag="den2")
                nc.vector.tensor_add(out=den2, in0=den_t, in1=dcor_s)
                rden = io.tile([128, QC], F32, tag="rden")
                nc.vector.reciprocal(rden, den2)

            # ---- projection + residual + MLP tail ----
            with tc.tile_pool(name="ps_tail", bufs=1, space="PSUM") as pst, \
                 tc.tile_pool(name="ps_tail2", bufs=1, space="PSUM") as pst2, \
                 tc.tile_pool(name="tailw", bufs=2) as tailw:
                pz = pst.tile([128, QC], F32, tag="pz")
                for h in range(HEADS):
                    rdb = pst2.tile([128, QC], F32, tag="rdb")
                    nc.tensor.matmul(rdb,
                                     ones_rows_f[32 * h:32 * h + 1, :],
                                     rden[32 * h:32 * h + 1, :],
                                     start=True, stop=True,
                                     tile_position=(32 * h, 0))
                    ab = tailw.tile([128, QC], BF16, tag="ab")
                    nc.gpsimd.tensor_add(out=ab, in0=A_sb[:, h, :], in1=B_sb)
                    onh = tailw.tile([128, QC], BF16, tag="onh")
                    nc.vector.tensor_mul(out=onh, in0=ab, in1=rdb)
                    nc.tensor.matmul(pz, WCT_s[:, h, :], onh,
                                     start=(h == 0), stop=(h == HEADS - 1))
                z0 = io.tile([D, QC], F32, tag="z0")
                nc.scalar.activation(z0, pz, AF.Identity, bias=bproj_s)
                sk = io.tile([D, QC], F32, tag="sk")
                nc.sync.dma_start(sk, skipT[...])
                z = io.tile([D, QC], F32, tag="z")
                nc.vector.tensor_add(out=z, in0=z0, in1=sk)

                zf = io.tile([D, QC], F32, tag="zf")
                _ln_partition(nc, work, pst2, ones_col, ones_row, eps_c,
                              z, zf, preg_s, preb_s)
                zfb = io.tile([D, QC], BF16, tag="zfb")
                nc.any.tensor_copy(out=zfb, in_=zf)

                x1 = io.tile([D, 2, QC], BF16, tag="x1")
                for j in range(2):
                    ph = pst2.tile([128, QC], F32, tag="ph")
                    nc.tensor.matmul(ph, w1_s[:, 128 * j:128 * (j + 1)], zfb,
                                     start=True, stop=True)
                    nc.scalar.activation(x1[:, j, :], ph, AF.Identity,
                                         bias=b1_s[:, j:j + 1])
                # gelu(x) = x*(0.5 + x*(GA + GB*y + GC*y^2)), y=x^2 (gpsimd)
                y1 = io.tile([D, 2, QC], BF16, tag="y1")
                nc.vector.tensor_mul(out=y1, in0=x1, in1=x1)
                p1 = io.tile([D, 2, QC], BF16, tag="p1")
                nc.vector.tensor_scalar(out=p1, in0=y1, scalar1=GC,
                                        scalar2=GB, op0=ALU.mult, op1=ALU.add)
                p2 = io.tile([D, 2, QC], BF16, tag="p2")
                nc.vector.tensor_mul(out=p2, in0=p1, in1=y1)
                p3 = io.tile([D, 2, QC], BF16, tag="p3")
                nc.vector.tensor_scalar(out=p3, in0=p2, scalar1=GA,
                                        scalar2=None, op0=ALU.add)
                p4 = io.tile([D, 2, QC], BF16, tag="p4")
                nc.vector.tensor_mul(out=p4, in0=p3, in1=x1)
                p5 = io.tile([D, 2, QC], BF16, tag="p5")
                nc.vector.tensor_scalar(out=p5, in0=p4, scalar1=0.5,
                                        scalar2=None, op0=ALU.add)
                h1 = io.tile([D, 2, QC], BF16, tag="h1")
                nc.vector.tensor_mul(out=h1, in0=p5, in1=x1)

                pm = pst.tile([128, QC], F32, tag="pm")
                nc.tensor.matmul(pm, w2_s[:, 0, :], h1[:, 0, :], start=True,
                                 stop=False)
                nc.tensor.matmul(pm, w2_s[:, 1, :], h1[:, 1, :], start=False,
                                 stop=True)
                z2 = io.tile([D, QC], F32, tag="z2")
                nc.scalar.activation(z2, pm, AF.Identity, bias=b2_s)
                z3 = io.tile([D, QC], F32, tag="z3")
                nc.vector.tensor_add(out=z3, in0=z2, in1=zf)

                zo = io.tile([D, QC], F32, tag="zo")
                _ln_partition(nc, work, pst2, ones_col, ones_row, eps_c,
                              z3, zo, postg_s, postb_s)
                nc.sync.dma_start(outT[...], zo)

    nc.finalize()
    _CACHE["nc"] = nc
    return nc


def _prep_inputs(inputs):
    f32 = np.float32
    bf16 = ml_dtypes.bfloat16
    q = np.asarray(inputs["q"], f32)
    k = np.asarray(inputs["k"], f32)
    v = np.asarray(inputs["v"], f32)
    W = np.asarray(inputs["W_logits"], f32)
    vis = np.asarray(inputs["vis"])
    skip = np.asarray(inputs["skip"], f32)

    g = lambda n: np.asarray(inputs[n], f32)
    qn_g, qn_b = g("qn_g"), g("qn_b")
    kn_g, kn_b = g("kn_g"), g("kn_b")
    vn_g, vn_b = g("vn_g"), g("vn_b")
    wq, bq = g("wq"), g("bq")
    wk, bk = g("wk"), g("bk")
    wv, bv = g("wv"), g("bv")
    wproj, bproj = g("wproj"), g("bproj")
    pre_g, pre_b = g("pre_g"), g("pre_b")
    w1, b1 = g("w1"), g("b1")
    w2, b2 = g("w2"), g("b2")
    post_g, post_b = g("post_g"), g("post_b")

    wq2 = (wq * qn_g[None, :]) * SCALE
    bq2 = (wq @ qn_b + bq) * SCALE
    wk2 = wk * kn_g[None, :]
    bk2 = wk @ kn_b + bk
    assert np.abs(bk2).max() < 1e-6, "nonzero k-bias not supported"
    wv2 = wv * vn_g[None, :]
    bv2 = wv @ vn_b + bv

    WkC = wk2 - wk2.mean(axis=1, keepdims=True)          # [inner, D]
    WCT = np.zeros((D, HEADS, D), f32)
    for h in range(HEADS):
        WC_h = wproj[:, 32 * h:32 * h + 32] @ wv2[32 * h:32 * h + 32, :]
        WCT[:, h, :] = WC_h.T
    bprojv = (wproj @ bv2 + bproj)[:, None]

    qTf = np.zeros((D, QPAD), f32)
    qTf[:, :Q] = q.reshape(D, Q)
    skipTf = np.zeros((D, QPAD), f32)
    skipTf[:, :Q] = skip.reshape(D, Q)

    kT = np.zeros((D, NKP), f32)
    kT[:, :NK] = np.transpose(k, (0, 2, 1, 3, 4)).reshape(D, NK)
    kRow = np.zeros((NKP, D), f32)
    kRow[:NK] = np.transpose(k, (0, 1, 3, 4, 2)).reshape(NK, D)
    vRow = np.zeros((NKP, D), f32)
    vRow[:NK] = np.transpose(v, (0, 1, 3, 4, 2)).reshape(NK, D)

    visf = vis[0].astype(f32)
    Wp = np.zeros((QPAD, NKP), f32)
    Wp[:Q, :NK] = W[0] * visf
    Vm1 = np.zeros((QPAD, NKP), f32)
    Vm1[:Q, :NK] = visf - 1.0
    Vm1[:Q, NK:] = -1.0
    dencorr = Vm1.sum(axis=1)
    dcor = np.zeros((D, QPAD), f32)
    for h in range(HEADS):
        dcor[32 * h, :] = dencorr

    shared = {
        "kTd": kT.astype(bf16),
        "kRd": kRow.astype(bf16),
        "vRd": vRow.astype(bf16),
        "Wq2Td": np.ascontiguousarray(wq2.T).astype(bf16),
        "bq2d": np.ascontiguousarray(bq2[:, None]),
        "WkCTd": np.ascontiguousarray(WkC.T).astype(bf16),
        "WCTd": WCT.astype(bf16),
        "bprojd": np.ascontiguousarray(bprojv),
        "pre_gd": np.ascontiguousarray(pre_g[:, None]),
        "pre_bd": np.ascontiguousarray(pre_b[:, None]),
        "w1Td": np.ascontiguousarray(w1.T).astype(bf16),
        "b1d": np.ascontiguousarray(b1.reshape(2, D).T),
        "w2Tdd": np.ascontiguousarray(w2.T.reshape(2, D, D)).astype(bf16),
        "b2d": np.ascontiguousarray(b2[:, None]),
        "post_gd": np.ascontiguousarray(post_g[:, None]),
        "post_bd": np.ascontiguousarray(post_b[:, None]),
    }

    in_maps = []
    for c in range(NCORES):
        sl = slice(c * QC, (c + 1) * QC)
        m = dict(shared)
        m["qT"] = np.ascontiguousarray(qTf[:, sl])
        m["skipT"] = np.ascontiguousarray(skipTf[:, sl])
        m["Wvd"] = np.ascontiguousarray(
            Wp[sl].T).reshape(NKT, 128, QC).astype(bf16)
        m["Vm1d"] = np.ascontiguousarray(
            Vm1[sl].T).reshape(NKT, 128, QC).astype(bf16)
        m["dcord"] = np.ascontiguousarray(dcor[:, sl])
        in_maps.append(m)
    return in_maps


def kernel(**inputs):
    from concourse.bass_utils import run_bass_kernel_spmd

    nc = _build()
    in_maps = _prep_inputs(inputs)
    res = run_bass_kernel_spmd(nc, in_maps, core_ids=list(range(NCORES)))
    outs = np.concatenate([r["outT"] for r in res.results], axis=1)
    return outs[:, :Q].reshape(1, D, HB, WB).astype(np.float32)
